# revision 9
# baseline (speedup 1.0000x reference)
"""Trainium2 Bass kernel for nn_Bert segment-mean (segment_reduce).

out[b, w, :] = mean(emb[b, st:ed, :]) if (mask != 0 and ed > st) else 0

Full shapes: emb [64, 512, 1024] f32, offsets [64, 400, 2] i32, mask [64, 400] i32.
Data-parallel over batch: 8 rows per core on 8 NeuronCores.

Per-core program (R=8 rows), per row:
  span[s, w] = (st_w <= s) * (s < ed_w)      built on DVE in [s-partition, w-free]
  psum[w, d] = sum_k span_k[:, w].T @ emb_k[:, d]   fp32r matmuls, fp32 PSUM accum
  out[w, d]  = psum[w, d] * scale_w          ScalarE activation(Copy, scale), where
  scale_w    = valid_w / max(ed_w - st_w, 1) precomputed on host (O(B*W) scalars)
"""

import os
import sys

for _p in ("/opt/trn_rl_repo", "/root/.axon_site/_ro/trn_rl_repo"):
    if os.path.isdir(_p) and _p not in sys.path:
        sys.path.insert(0, _p)

import numpy as np

import concourse.bacc as bacc
import concourse.mybir as mybir
import concourse.tile as tile
from concourse.bass_utils import run_bass_kernel_spmd

B, S, W, D = 64, 512, 400, 1024
N_CORES = 8
R = B // N_CORES          # batch rows per core
KC = S // 128             # contraction chunks (4)
MC = (W + 127) // 128     # output w chunks (4; last is 16 wide)
NC_ = D // 512            # moving-dim chunks (2)

f32 = mybir.dt.float32
f32r = mybir.dt.float32r
i32 = mybir.dt.int32

# Results of the most recent run, for test harnesses.
LAST_RESULTS = None


def build_program(rows=R):
    nc = bacc.Bacc("TRN2", target_bir_lowering=False, debug=False)

    emb_d = nc.dram_tensor("emb", [rows, S, D], f32, kind="ExternalInput").ap()
    st_d = nc.dram_tensor("st", [rows, W], f32, kind="ExternalInput").ap()
    ed_d = nc.dram_tensor("ed", [rows, W], f32, kind="ExternalInput").ap()
    # scale[r, p, m] = valid/max(len,1) for word m*128+p (host-transposed, padded)
    scale_d = nc.dram_tensor("scale", [rows, 128, MC], f32, kind="ExternalInput").ap()
    out_d = nc.dram_tensor("out", [rows, W, D], f32, kind="ExternalOutput").ap()

    with tile.TileContext(nc) as tc:
        with (
            tc.tile_pool(name="const", bufs=1) as constp,
            tc.tile_pool(name="emb", bufs=2) as embp,
            tc.tile_pool(name="span", bufs=2) as spanp,
            tc.tile_pool(name="bcast", bufs=2) as bcastp,
            tc.tile_pool(name="scale", bufs=2) as scalep,
            tc.tile_pool(name="outs", bufs=3) as outp,
            tc.tile_pool(name="psum", bufs=8, space="PSUM") as psump,
        ):
            # iota[p, k] = 128*k + p  (the s index of partition p in chunk k;
            # values < 2^24 so exact in f32)
            iota_t = constp.tile([128, KC], f32)
            for k in range(KC):
                nc.gpsimd.iota(
                    iota_t[:, k : k + 1],
                    pattern=[[0, 1]],
                    base=128 * k,
                    channel_multiplier=1,
                    allow_small_or_imprecise_dtypes=True,
                )

            for r in range(rows):
                emb_t = embp.tile([128, KC, D], f32)
                nc.sync.dma_start(
                    out=emb_t[:], in_=emb_d[r].rearrange("(k p) d -> p k d", p=128)
                )
                # fp32r matmul operands must be rounded by a compute op
                embr_t = embp.tile([128, KC, D], f32r, tag="embr")
                nc.vector.tensor_copy(embr_t[:], emb_t[:])

                stb = bcastp.tile([128, W], f32, tag="stb")
                edb = bcastp.tile([128, W], f32, tag="edb")
                nc.sync.dma_start(out=stb[:], in_=st_d[r].partition_broadcast(128))
                nc.sync.dma_start(out=edb[:], in_=ed_d[r].partition_broadcast(128))

                scale_t = scalep.tile([128, MC], f32)
                nc.sync.dma_start(out=scale_t[:], in_=scale_d[r])

                span_t = spanp.tile([128, KC, W], f32r)
                for k in range(KC):
                    a_t = spanp.tile([128, W], f32, tag="ge")
                    b_t = spanp.tile([128, W], f32, tag="lt")
                    # a = (st <= s), b = (ed > s), span = a * b
                    nc.vector.tensor_scalar(
                        a_t[:], stb[:], iota_t[:, k : k + 1], None, mybir.AluOpType.is_le
                    )
                    nc.vector.tensor_scalar(
                        b_t[:], edb[:], iota_t[:, k : k + 1], None, mybir.AluOpType.is_gt
                    )
                    nc.vector.tensor_tensor(
                        span_t[:, k, :], a_t[:], b_t[:], mybir.AluOpType.mult
                    )

                for m in range(MC):
                    m0 = m * 128
                    mw = min(128, W - m0)
                    out_t = outp.tile([128, D], f32)
                    for n in range(NC_):
                        n0 = n * 512
                        ps = psump.tile([128, 512], f32)
                        for k in range(KC):
                            nc.tensor.matmul(
                                ps[:mw, :],
                                span_t[:, k, m0 : m0 + mw],
                                embr_t[:, k, n0 : n0 + 512],
                                start=(k == 0),
                                stop=(k == KC - 1),
                            )
                        nc.scalar.activation(
                            out_t[:mw, n0 : n0 + 512],
                            ps[:mw, :],
                            mybir.ActivationFunctionType.Copy,
                            scale=scale_t[:mw, m : m + 1],
                        )
                    nc.sync.dma_start(
                        out=out_d[r, m0 : m0 + mw, :], in_=out_t[:mw, :]
                    )

    nc.compile()
    return nc


def host_prep(bert_embedding, x_bert_offset, x_mask):
    """Split inputs into per-core input maps."""
    st = np.ascontiguousarray(x_bert_offset[..., 0]).astype(np.int32)
    ed = np.ascontiguousarray(x_bert_offset[..., 1]).astype(np.int32)
    stf = st.astype(np.float32)
    edf = ed.astype(np.float32)
    lens = (ed - st).astype(np.float32)
    valid = (x_mask != 0) & (ed > st)
    scale = np.where(valid, 1.0 / np.maximum(lens, 1.0), 0.0).astype(np.float32)
    # transpose to [b, p, m] with w = m*128 + p, zero-padded to MC*128
    scale_pad = np.zeros((B, MC * 128), np.float32)
    scale_pad[:, :W] = scale
    scale_t = np.ascontiguousarray(
        scale_pad.reshape(B, MC, 128).transpose(0, 2, 1)
    )

    emb = np.ascontiguousarray(bert_embedding, dtype=np.float32)
    in_maps = []
    for c in range(N_CORES):
        sl = slice(c * R, (c + 1) * R)
        in_maps.append(
            {
                "emb": emb[sl],
                "st": np.ascontiguousarray(stf[sl]),
                "ed": np.ascontiguousarray(edf[sl]),
                "scale": np.ascontiguousarray(scale_t[sl]),
            }
        )
    return in_maps


_PROGRAM_CACHE = {}


def kernel(bert_embedding, x_bert_offset, x_mask, trace=False):
    global LAST_RESULTS
    assert bert_embedding.shape == (B, S, D), bert_embedding.shape
    if R not in _PROGRAM_CACHE:
        _PROGRAM_CACHE[R] = build_program(R)
    nc = _PROGRAM_CACHE[R]
    in_maps = host_prep(bert_embedding, x_bert_offset, x_mask)
    res = run_bass_kernel_spmd(nc, in_maps, list(range(N_CORES)), trace=trace)
    LAST_RESULTS = res
    out = np.concatenate([res.results[c]["out"] for c in range(N_CORES)], axis=0)
    return out.astype(np.float32)


# revision 11
# speedup vs baseline: 1.5634x; 1.5634x over previous
"""Trainium2 Bass kernel for nn_Bert segment-mean (segment_reduce).

out[b, w, :] = mean(emb[b, st:ed, :]) if (mask != 0 and ed > st) else 0

Full shapes: emb [64, 512, 1024] f32, offsets [64, 400, 2] i32, mask [64, 400] i32.
Data-parallel over batch: 8 rows per core on 8 NeuronCores.

Per-core program (R=8 rows), per row:
  span[s, w] = (st_w <= s) * (s < ed_w)      built on DVE in [s-partition, w-free]
  psum[w, d] = sum_k span_k[:, w].T @ emb_k[:, d]   matmuls, fp32 PSUM accum
  out[w, d]  = psum[w, d] * scale_w          ScalarE activation(Copy, scale), where
  scale_w    = valid_w / max(ed_w - st_w, 1) precomputed on host (O(B*W) scalars)

MM_DTYPE selects the matmul operand dtype:
  bf16 — emb cast to bf16 on host (halves input DMA), 1 cyc/row matmuls, N=1024
  f32r — emb shipped f32, rounded to f32r on DVE, N=512 matmuls (~2x PE time,
         ~8x lower numeric error)
"""

import os
import sys

for _p in ("/opt/trn_rl_repo", "/root/.axon_site/_ro/trn_rl_repo"):
    if os.path.isdir(_p) and _p not in sys.path:
        sys.path.insert(0, _p)

import numpy as np

import concourse.bacc as bacc
import concourse.mybir as mybir
import concourse.tile as tile
from concourse.bass_utils import run_bass_kernel_spmd

B, S, W, D = 64, 512, 400, 1024
N_CORES = 8
R = B // N_CORES          # batch rows per core
KC = S // 128             # contraction chunks (4)
MC = (W + 127) // 128     # output w chunks (4; last is 16 wide)

f32 = mybir.dt.float32
f32r = mybir.dt.float32r
bf16 = mybir.dt.bfloat16
i32 = mybir.dt.int32

MM_DTYPE = os.environ.get("BERT_MM_DTYPE", "bf16")

# Results of the most recent run, for test harnesses.
LAST_RESULTS = None


def build_program(rows=R, mm_dtype=None):
    mm_dtype = mm_dtype or MM_DTYPE
    use_bf16 = mm_dtype == "bf16"
    mdt = bf16 if use_bf16 else f32r
    NW = 512                  # matmul moving-dim width (PSUM bank = 512 fp32)
    NN = D // NW

    nc = bacc.Bacc("TRN2", target_bir_lowering=False, debug=False)

    emb_d = nc.dram_tensor(
        "emb", [rows, S, D], bf16 if use_bf16 else f32, kind="ExternalInput"
    ).ap()
    st_d = nc.dram_tensor("st", [rows, W], f32, kind="ExternalInput").ap()
    ed_d = nc.dram_tensor("ed", [rows, W], f32, kind="ExternalInput").ap()
    # scale[r, p, m] = valid/max(len,1) for word m*128+p (host-transposed, padded)
    scale_d = nc.dram_tensor("scale", [rows, 128, MC], f32, kind="ExternalInput").ap()
    out_d = nc.dram_tensor("out", [rows, W, D], f32, kind="ExternalOutput").ap()

    with tile.TileContext(nc) as tc:
        with (
            tc.tile_pool(name="const", bufs=1) as constp,
            tc.tile_pool(name="emb", bufs=4 if use_bf16 else 2) as embp,
            tc.tile_pool(name="rows", bufs=4) as rowp,
            tc.tile_pool(name="span", bufs=3) as spanp,
            tc.tile_pool(name="bcast", bufs=3) as bcastp,
            tc.tile_pool(name="scale", bufs=4) as scalep,
            tc.tile_pool(name="outs", bufs=4) as outp,
            tc.tile_pool(name="psum", bufs=4, space="PSUM") as psump,
        ):
            # iota[p, k] = 128*k + p  (the s index of partition p in chunk k;
            # values < 2^24 so exact in f32)
            iota_t = constp.tile([128, KC], f32)
            for k in range(KC):
                nc.gpsimd.iota(
                    iota_t[:, k : k + 1],
                    pattern=[[0, 1]],
                    base=128 * k,
                    channel_multiplier=1,
                    allow_small_or_imprecise_dtypes=True,
                )

            for r in range(rows):
                # small loads first so mask building is never behind emb bytes
                st_row = rowp.tile([1, W], f32, tag="strow")
                ed_row = rowp.tile([1, W], f32, tag="edrow")
                scale_t = scalep.tile([128, MC], f32)
                nc.sync.dma_start(out=st_row[:], in_=st_d[r : r + 1, :])
                nc.sync.dma_start(out=ed_row[:], in_=ed_d[r : r + 1, :])
                nc.sync.dma_start(out=scale_t[:], in_=scale_d[r])

                emb_t = embp.tile([128, KC, D], bf16 if use_bf16 else f32)
                nc.sync.dma_start(
                    out=emb_t[:], in_=emb_d[r].rearrange("(k p) d -> p k d", p=128)
                )
                if use_bf16:
                    embr_t = emb_t
                else:
                    # fp32r matmul operands must be rounded by a compute op
                    embr_t = embp.tile([128, KC, D], f32r, tag="embr")
                    nc.vector.tensor_copy(embr_t[:], emb_t[:])

                stb = bcastp.tile([128, W], f32, tag="stb")
                edb = bcastp.tile([128, W], f32, tag="edb")
                nc.gpsimd.partition_broadcast(stb[:], st_row[:])
                nc.gpsimd.partition_broadcast(edb[:], ed_row[:])

                span_t = spanp.tile([128, KC, W], mdt)
                for k in range(KC):
                    a_t = spanp.tile([128, W], mdt, tag="ge")
                    b_t = spanp.tile([128, W], mdt, tag="lt")
                    # a = (st <= s), b = (ed > s), span = a * b
                    nc.vector.tensor_scalar(
                        a_t[:], stb[:], iota_t[:, k : k + 1], None, mybir.AluOpType.is_le
                    )
                    nc.vector.tensor_scalar(
                        b_t[:], edb[:], iota_t[:, k : k + 1], None, mybir.AluOpType.is_gt
                    )
                    nc.vector.tensor_tensor(
                        span_t[:, k, :], a_t[:], b_t[:], mybir.AluOpType.mult
                    )

                for m in range(MC):
                    m0 = m * 128
                    mw = min(128, W - m0)
                    out_t = outp.tile([128, D], f32)
                    ps = psump.tile([128, D], f32)
                    for n in range(NN):
                        n0 = n * NW
                        for k in range(KC):
                            nc.tensor.matmul(
                                ps[:mw, n0 : n0 + NW],
                                span_t[:, k, m0 : m0 + mw],
                                embr_t[:, k, n0 : n0 + NW],
                                start=(k == 0),
                                stop=(k == KC - 1),
                            )
                    nc.scalar.activation(
                        out_t[:mw, :],
                        ps[:mw, :],
                        mybir.ActivationFunctionType.Copy,
                        scale=scale_t[:mw, m : m + 1],
                    )
                    # store triggered from ScalarE: ordered after its own
                    # ACTIVATE, keeps the sync queue free for loads
                    nc.scalar.dma_start(
                        out=out_d[r, m0 : m0 + mw, :], in_=out_t[:mw, :]
                    )

    nc.compile()
    return nc


def host_prep(bert_embedding, x_bert_offset, x_mask, use_bf16):
    """Split inputs into per-core input maps."""
    st = np.ascontiguousarray(x_bert_offset[..., 0]).astype(np.int32)
    ed = np.ascontiguousarray(x_bert_offset[..., 1]).astype(np.int32)
    stf = st.astype(np.float32)
    edf = ed.astype(np.float32)
    lens = (ed - st).astype(np.float32)
    valid = (x_mask != 0) & (ed > st)
    scale = np.where(valid, 1.0 / np.maximum(lens, 1.0), 0.0).astype(np.float32)
    # transpose to [b, p, m] with w = m*128 + p, zero-padded to MC*128
    scale_pad = np.zeros((B, MC * 128), np.float32)
    scale_pad[:, :W] = scale
    scale_t = np.ascontiguousarray(
        scale_pad.reshape(B, MC, 128).transpose(0, 2, 1)
    )

    emb = np.ascontiguousarray(bert_embedding, dtype=np.float32)
    if use_bf16:
        import ml_dtypes

        emb = emb.astype(ml_dtypes.bfloat16)
    in_maps = []
    for c in range(N_CORES):
        sl = slice(c * R, (c + 1) * R)
        in_maps.append(
            {
                "emb": emb[sl],
                "st": np.ascontiguousarray(stf[sl]),
                "ed": np.ascontiguousarray(edf[sl]),
                "scale": np.ascontiguousarray(scale_t[sl]),
            }
        )
    return in_maps


_PROGRAM_CACHE = {}


def kernel(bert_embedding, x_bert_offset, x_mask, trace=False):
    global LAST_RESULTS
    assert bert_embedding.shape == (B, S, D), bert_embedding.shape
    key = (R, MM_DTYPE)
    if key not in _PROGRAM_CACHE:
        _PROGRAM_CACHE[key] = build_program(R, MM_DTYPE)
    nc = _PROGRAM_CACHE[key]
    in_maps = host_prep(bert_embedding, x_bert_offset, x_mask, MM_DTYPE == "bf16")
    res = run_bass_kernel_spmd(nc, in_maps, list(range(N_CORES)), trace=trace)
    LAST_RESULTS = res
    out = np.concatenate([res.results[c]["out"] for c in range(N_CORES)], axis=0)
    return out.astype(np.float32)
